# revision 19
# baseline (speedup 1.0000x reference)
"""BottleneckAttention TRN2 kernel: 8 NeuronCores, one (batch, head) pair per core.

Decomposition (per core, batch b / head i):
  q = (scale * Wq_i) @ x_b          [64, 4096]   (d-major)
  k = Wk_i @ x_b                    [64, 4096]
  vT = (Wv_i @ x_b)^T               [4096, 64]   (n-major, built chunkwise)
  Height rel-bias folded into the score matmul via an augmented contraction:
     K_aug = [k; Ih]  Q_aug = [q; RH^T]     (Ih[h',j] = 1 if j//64 == h')
     S^T[j,q] = K_aug^T Q_aug = content + height-bias   (bf16 PSUM)
  Width rel-bias applied after exp via the separability of exp. Two paths,
  chosen per key-chunk so the scalar (ACT) and vector (DVE) engines stay
  balanced -- ACT's exp throughput alone would pace the whole loop:
    classic (3 of 4 chunks): e = exp(S^T) * ew_dup   (ACT exp + DVE mul)
    fast-exp (1 of 4):       Schraudolph in bf16 bit space, DVE only:
       e16 = int16(S^T * 128*log2e + LW),  LW = 128*log2e * RW^T + C
       (one fused scalar_tensor_tensor; the int16 bits ARE the bf16
       approximation of exp(S^T + RW^T), width bias folded in).
  PV + row-sums fused: vT_aug = [vT | 1] so out rows 0..63 = unnormalized
  attention output (transposed [d, q]), row 64 = softmax denominators.
  The softmax reciprocal runs on a DMA-transposed [128, 8] layout (a 1-lane
  [1,1024] reciprocal would block the DVE queue for ~6.7us).
  Output projection partial: P = Wout[:, i*64:(i+1)*64] @ out, then columns
  scaled by 1/sums (normalization commutes with the d-contraction).
Host sums the 4 per-head partials per batch and adds the residual x.

All inputs arrive pre-cast to bf16 from the host (the matmuls consumed bf16
anyway): no on-device casts, half the input DMA. Epilogue PSUM-side copies
run on the ACT engine (gpsimd cannot touch PSUM, and big gpsimd tensor ops
degrade concurrent DVE throughput via SBUF port contention).

Softmax skips the max-subtraction (logits stay within ~[-12, 12]).
"""

import numpy as np
import ml_dtypes

import concourse.bass as bass
import concourse.bacc as bacc
import concourse.tile as tile
from concourse import mybir
from concourse.bass_utils import run_bass_kernel_spmd

F32 = mybir.dt.float32
BF16 = mybir.dt.bfloat16
I16 = mybir.dt.int16
AF = mybir.ActivationFunctionType
ALU = mybir.AluOpType

HEADS, B, C, HH, WW = 4, 2, 256, 64, 64
N = HH * WW           # 4096
DH = C // HEADS       # 64
NQ = 4                # query blocks
QB = N // NQ          # 1024 query cols per block
NJC = 32              # key chunks of 128
NWARM = 8             # PE p-state warmup matmuls (overlap the input DMA)
SCHR_A = 128.0 / float(np.log(2.0))      # bf16-bit-space log2(e) scale
SCHR_C = 128.0 * 127.0 - 5.5             # exponent bias - Schraudolph offset


def _body(tc, io):
    from contextlib import ExitStack
    with ExitStack() as ctx:
        _body_inner(tc, io, ctx)


def _body_inner(tc, io, ctx):
    nc = tc.nc
    xb, wq, wk, wv, wo, relw, relh, ih, out = (
        io["xb"], io["wq"], io["wk"], io["wv"], io["wo"],
        io["relw"], io["relh"], io["ih"], io["out"],
    )

    big = ctx.enter_context(tc.tile_pool(name="big", bufs=1))
    rot = ctx.enter_context(tc.tile_pool(name="rot", bufs=8))
    ep = ctx.enter_context(tc.tile_pool(name="ep", bufs=2))
    spool = ctx.enter_context(tc.tile_pool(name="spool", bufs=2, space="PSUM"))
    opool = ctx.enter_context(tc.tile_pool(name="opool", bufs=2, space="PSUM"))
    dpool = ctx.enter_context(tc.tile_pool(name="dpool", bufs=2, space="DRAM"))

    # ---- input DMAs (all inputs already bf16) ----------------------
    # xb gates the q builds which gate everything; it goes first on both
    # queues. Small weights follow behind.
    wq_bf = big.tile([128, 2, DH], BF16)
    wk_bf = big.tile([128, 2, DH], BF16)
    wv_bf = big.tile([128, 2, DH], BF16)
    nc.sync.dma_start(out=wq_bf, in_=wq.rearrange("(cc p) d -> p cc d", p=128))

    K_aug = big.tile([128, N], BF16)
    Q_aug = big.tile([128, N], BF16)
    ew_dup = big.tile([128, N], BF16)
    lw32 = big.tile([128, N], F32)
    rwt = big.tile([64, N], BF16)
    vt_aug = big.tile([128, NJC, 65], BF16)
    h_sb = big.tile([64, N], BF16)

    xb_bf = big.tile([128, 2, N], BF16)
    xv = xb.rearrange("(cc p) n -> p cc n", p=128)
    # DMAs can issue from the sync, gpsimd and scalar queues. Fine-grained
    # chunks in column-major order make the early slabs (which gate the q
    # and k/v builds) complete first instead of round-robin-finishing all
    # at once with the tail of the tensor.
    relw_bf = big.tile([64, 127], BF16)
    relh_bf = big.tile([64, 127], BF16)
    wo_bf = big.tile([64, 256], BF16)
    qs = [nc.sync, nc.gpsimd, nc.scalar]
    qi = 0
    for s8 in range(8):
        for cc in range(2):
            eng = qs[qi % 3]
            qi += 1
            eng.dma_start(out=xb_bf[:, cc, bass.ts(s8, N // 8)],
                          in_=xv[:, cc, bass.ts(s8, N // 8)])
        if s8 == 1:
            nc.gpsimd.dma_start(
                out=wk_bf, in_=wk.rearrange("(cc p) d -> p cc d", p=128))
            nc.scalar.dma_start(
                out=wv_bf, in_=wv.rearrange("(cc p) d -> p cc d", p=128))
    nc.sync.dma_start(out=relw_bf, in_=relw)
    nc.sync.dma_start(out=relh_bf, in_=relh)
    nc.gpsimd.dma_start(out=wo_bf, in_=wo)
    # Ih rows of K_aug last: not needed until the first S matmul, and it
    # must not steal DMA bandwidth from xb
    nc.scalar.dma_start(out=K_aug[64:128, :], in_=ih)

    # PE warm-up: independent dummy matmuls ramp the PE p-state while
    # the input DMA lands, so real matmuls start at the warm clock.
    warm = big.tile([128, 512], BF16)
    nc.vector.memset(warm, 0.0)
    ones1 = big.tile([1, 128], F32)
    nc.vector.memset(ones1, 1.0)
    wps = spool.tile([128, 512], F32, tag="sp")
    for _ in range(NWARM):
        nc.tensor.matmul(wps, warm[:, 0:128], warm, start=True, stop=True)

    def act_copy(out, in_):
        nc.scalar.activation(out=out, in_=in_, func=AF.Copy)

    def qk_build(dst, w_bf, qq):
        ps = spool.tile([128, QB], F32, tag="sp")
        for cc in range(2):
            for h in range(2):
                nc.tensor.matmul(
                    ps[0:64, bass.ts(h, 512)],
                    w_bf[:, cc, :],
                    xb_bf[:, cc, qq * QB + h * 512: qq * QB + (h + 1) * 512],
                    start=(cc == 0), stop=(cc == 1),
                )
        act_copy(out=dst[0:64, bass.ts(qq, QB)], in_=ps[0:64, :])

    q_xy = Q_aug[0:64, :].rearrange("d (x y) -> d x y", y=64)
    rwt_xy = rwt.rearrange("jw (x y) -> jw x y", y=64)

    def rh_build(g, ps=None):
        # RH^T[jh, n=(x,y)] = sum_d relh[jh - x + 63, d] * q[d, n]
        if ps is None:
            ps = spool.tile([128, QB], F32, tag="sp")
        for xi in range(16):
            xx = g * 16 + xi
            nc.tensor.matmul(
                ps[0:64, bass.ts(xi, 64)],
                relh_bf[:, 63 - xx: 127 - xx],
                Q_aug[0:64, xx * 64: (xx + 1) * 64],
                start=True, stop=True,
            )
        act_copy(out=Q_aug[64:128, bass.ts(g, QB)], in_=ps[0:64, :])

    def rw_build(g):
        # RW^T[jw, n=(x,y)] = sum_d relw[jw - y + 63, d] * q[d, n]
        ps = spool.tile([128, QB], F32, tag="sp")
        for yi in range(16):
            yy = g * 16 + yi
            nc.tensor.matmul(
                ps[0:64, bass.ts(yi, 64)],
                relw_bf[:, 63 - yy: 127 - yy],
                q_xy[:, :, yy],
                start=True, stop=True,
            )
        # ps free layout is [yi, x]; rwt quarter slice wants [x, y].
        # One strided copy with a transposed view of the psum tile.
        nc.vector.tensor_copy(
            out=rwt_xy[:, :, g * 16:(g + 1) * 16],
            in_=ps[0:64, :].rearrange("p (yi x) -> p x yi", x=64))

    def vt_build(g):
        ps = spool.tile([128, 8, 64], F32, tag="sp")
        for ci in range(8):
            chunk = g * 8 + ci
            for cc in range(2):
                nc.tensor.matmul(
                    ps[:, ci, :],
                    xb_bf[:, cc, chunk * 128: (chunk + 1) * 128],
                    wv_bf[:, cc, :],
                    start=(cc == 0), stop=(cc == 1),
                )
        act_copy(out=vt_aug[:, g * 8: (g + 1) * 8, 0:64], in_=ps)

    # Pre-main builds: only what quarter 0 needs up-front. q feeds RW
    # (the ew/LWri wall), so q and RW come first; the remaining k/vT/RH
    # groups are injected into quarter 0's stream below.
    nc.vector.memset(vt_aug[:, :, 64:65], 1.0)
    for qq in range(NQ):
        qk_build(Q_aug, wq_bf, qq)
    ew_xy = ew_dup.rearrange("p (x y) -> p x y", y=64)
    lw_xy = lw32.rearrange("p (x y) -> p x y", y=64)
    for g in range(4):
        rw_build(g)
        # exp / int16-scale + row-duplication of this y-stripe right away
        # so the tables are ready when the last stripe lands (pipelines
        # with the rw PE work)
        sl = slice(g * 16, (g + 1) * 16)
        nc.scalar.activation(out=ew_xy[0:64, :, sl], in_=rwt_xy[:, :, sl],
                             func=AF.Exp)
        nc.vector.tensor_copy(out=ew_xy[64:128, :, sl], in_=ew_xy[0:64, :, sl])
        # lw = A*rw + C via ACT Copy's fused scale+bias; dup on DVE so the
        # prologue table work splits across both engines
        nc.scalar.activation(out=lw_xy[0:64, :, sl], in_=rwt_xy[:, :, sl],
                             func=AF.Copy, scale=SCHR_A, bias=SCHR_C)
        nc.vector.tensor_copy(out=lw_xy[64:128, :, sl], in_=lw_xy[0:64, :, sl])
    # k/vT/RH builds for the early chunks go AFTER the rw chain: rw gates
    # the first e-tile of the whole loop; these only gate S/PV chunks that
    # run a few steps in
    qk_build(K_aug, wk_bf, 0)
    rh_build(0)
    vt_build(0)
    qk_build(K_aug, wk_bf, 1)
    vt_build(1)

    # ---- main attention loop ---------------------------------------
    # Per query block: S^T matmul (PE, bf16 psum) -> softmax numerator
    # (ACT exp + DVE mul, or DVE-only fast-exp) -> PV (PE). PV emission
    # lags S by PVLAG stages so engine-queue hiccups never stall the
    # in-order PE stream. The previous block's epilogue is spread across
    # this block's stream in small pieces on ACT/DVE/sync.
    PVLAG = 6
    part1 = [None] * NQ             # per-quarter deferred epilogue pieces
    part2 = [None] * NQ

    def make_part1(qq, o_ps):
        rsb = ep.tile([128, QB], F32, tag="rsb")
        rd1 = dpool.tile([1, QB], F32, tag="rd1")
        rtp = ep.tile([128, 8], F32, tag="rtp")
        rtp2 = ep.tile([128, 8], F32, tag="rtp2")
        rd2 = dpool.tile([1, QB], F32, tag="rd2")
        rbc = ep.tile([128, QB], F32, tag="rbc")

        def row_copy():
            # sums row lives in one PSUM partition; only ACT/DVE can read
            # PSUM, and ACT is the idler engine here.
            act_copy(out=rsb[64:65, :], in_=o_ps[64:65, :])

        def bounce_out():
            nc.sync.dma_start(out=rd1, in_=rsb[64:65, :])
            nc.sync.dma_start(out=rtp, in_=rd1.rearrange("o (p e) -> (o p) e", p=128))

        def recip():
            nc.vector.reciprocal(out=rtp2, in_=rtp)

        def bounce_back():
            nc.sync.dma_start(out=rd2.rearrange("o (p e) -> (o p) e", p=128), in_=rtp2)
            nc.sync.dma_start(
                out=rbc,
                in_=bass.AP(tensor=rd2.tensor, offset=rd2.offset,
                            ap=[[0, 128]] + list(rd2.ap[1:])),
            )

        def h_copy(h):
            act_copy(out=h_sb[:, qq * QB + h * 512: qq * QB + (h + 1) * 512],
                     in_=o_ps[0:64, bass.ts(h, 512)])

        return [row_copy, bounce_out, recip, bounce_back,
                lambda: h_copy(0), lambda: h_copy(1)], rbc, rsb

    def make_part2(qq, rbc, o_ps):
        # proj matmuls and the normalize+store are separate pieces: the
        # osb multiply is only issued once its psum is already written, so
        # it never waits at the head of the in-order DVE queue. The proj
        # reuses this quarter's (fully consumed) o_ps psum buffer instead
        # of allocating from the S ring, so the next quarter's S matmuls
        # never contend with it.

        def proj_mm(oh):
            for h in range(2):
                nc.tensor.matmul(
                    o_ps[:, bass.ts(h, 512)],
                    wo_bf[:, oh * 128: (oh + 1) * 128],
                    h_sb[:, qq * QB + h * 512: qq * QB + (h + 1) * 512],
                    start=True, stop=True,
                )

        def osb_store(oh):
            osb = ep.tile([128, QB], BF16, tag="osb")
            nc.vector.tensor_mul(osb, o_ps, rbc)
            nc.sync.dma_start(
                out=out[oh * 128: (oh + 1) * 128, qq * QB: (qq + 1) * QB],
                in_=osb,
            )
        return proj_mm, osb_store

    MULLAG = 3
    for qq in range(NQ):
        o_ps = opool.tile([128, QB], F32)
        e_tiles = [None] * NJC
        e0_tiles = [None] * NJC

        def s_stage(jc):
            ps = spool.tile([128, QB], F32, tag="sp")
            for h in range(2):
                nc.tensor.matmul(
                    ps[:, bass.ts(h, 512)],
                    K_aug[:, jc * 128: (jc + 1) * 128],
                    Q_aug[:, qq * QB + h * 512: qq * QB + (h + 1) * 512],
                    start=True, stop=True,
                )
            schr = (jc % 3 == 2) if qq == 0 else (jc % 4 == 2)
            if schr:
                # DVE-only fast-exp: one fused (ps * A) + LW -> int16 op;
                # LW holds 128*log2e * RW^T + C in f32, so the int16
                # convert IS the bf16 bit pattern of exp(S^T + RW^T).
                e16 = rot.tile([128, QB], I16, tag="e")
                nc.vector.scalar_tensor_tensor(out=e16, in0=ps,
                                               scalar=SCHR_A,
                                               in1=lw32[:, bass.ts(qq, QB)],
                                               op0=ALU.mult, op1=ALU.add)
                e_tiles[jc] = e16.bitcast(BF16)
            else:
                e0 = rot.tile([128, QB], BF16, tag="e0")
                nc.scalar.activation(out=e0, in_=ps, func=AF.Exp)
                e0_tiles[jc] = e0

        def mul_stage(jc):
            # the ew multiply is issued MULLAG stages after its exp so the
            # in-order DVE queue never holds a not-yet-ready op at its
            # head (head-of-line blocking there stalls the psum recycling
            # and cascades into the PE stream)
            if e0_tiles[jc] is None:
                return
            e = rot.tile([128, QB], BF16, tag="e")
            nc.vector.tensor_mul(e, e0_tiles[jc], ew_dup[:, bass.ts(qq, QB)])
            e_tiles[jc] = e
            e0_tiles[jc] = None

        def pv_stage(jc):
            for h in range(2):
                nc.tensor.matmul(
                    o_ps[0:65, bass.ts(h, 512)],
                    vt_aug[:, jc, :],
                    e_tiles[jc][:, bass.ts(h, 512)],
                    start=(jc == 0), stop=(jc == NJC - 1),
                )
            e_tiles[jc] = None

        prev = part1[qq - 1][0] if qq > 0 else None
        for t in range(NJC + PVLAG):
            if t < NJC:
                s_stage(t)
            if qq == 0:
                if t == 2:
                    qk_build(K_aug, wk_bf, 2)
                elif t == 4:
                    vt_build(2)
                elif t == 8:
                    qk_build(K_aug, wk_bf, 3)
                elif t == 10:
                    vt_build(3)
                elif t == 20:
                    rh_build(1)
            else:
                if t == 6:
                    prev[0]()       # ACT: sums row PSUM -> SBUF
                elif t == 9:
                    prev[1]()       # DMA bounce out (transpose to [128,8])
                elif t == 12:
                    prev[2]()       # DVE reciprocal on [128,8] (~0.2us)
                elif t == 14:
                    prev[3]()       # DMA bounce back + broadcast
                elif t == 17:
                    prev[4]()       # ACT: h copy (first half)
                elif t == 20:
                    prev[5]()       # ACT: h copy (second half)
                elif t == 22 and qq <= 2:
                    # RH stripe for the NEXT quarter, into the previous
                    # quarter's now-dead o_ps psum (the S ring and the
                    # quarter boundary stay undisturbed)
                    rh_build(qq + 1, ps=part1[qq - 1][1])
                elif t == 25:
                    part2[qq - 1][0](0)     # proj matmuls oh=0
                elif t == 28:
                    part2[qq - 1][1](0)     # normalize + store oh=0
                elif t == 29:
                    part2[qq - 1][0](1)     # proj matmuls oh=1
                elif t == 32:
                    part2[qq - 1][1](1)     # normalize + store oh=1
            if t >= MULLAG and t - MULLAG < NJC:
                mul_stage(t - MULLAG)
            if t >= PVLAG:
                pv_stage(t - PVLAG)

        pieces, rbc, rsb_t = make_part1(qq, o_ps)
        part1[qq] = (pieces, o_ps, rsb_t)
        part2[qq] = make_part2(qq, rbc, o_ps)

    # Final quarter epilogue. The mid-loop DMA bounce (4 serial small
    # DMAs, ~2us completion latency each) would put ~8us of pure latency
    # on the tail, so the tail uses a latency-lean path instead: one
    # SBUF->SBUF partition-scatter DMA puts the sums on 8 lanes, the
    # reciprocal runs 8-wide, and PE matmuls against a ones column
    # broadcast the result across all 128 partitions.
    qq3 = NQ - 1
    p = part1[qq3][0]
    o_ps3 = part1[qq3][1]
    rsb3 = part1[qq3][2]
    p[0]()          # sums row -> SBUF (ACT)
    p[4]()          # h copy 0 (ACT)
    p[5]()          # h copy 1 (ACT)
    rst = ep.tile([8, 128], F32, tag="rtp")
    nc.sync.dma_start(out=rst, in_=rsb3[64:65, :])
    rst2 = ep.tile([8, 128], F32, tag="rtp2")
    nc.vector.reciprocal(out=rst2, in_=rst)
    rrow = ep.tile([1, QB], F32, tag="rrow")
    nc.sync.dma_start(out=rrow, in_=rst2)
    pm, ost = part2[qq3]
    pm(0)
    rbc_sb = []
    for half in range(2):
        rp = spool.tile([128, 512], F32, tag="sp")
        nc.tensor.matmul(rp, ones1, rrow[0:1, bass.ts(half, 512)],
                         start=True, stop=True)
        rs = ep.tile([128, 512], F32, tag="rbc3")
        nc.scalar.activation(out=rs, in_=rp, func=AF.Copy)
        rbc_sb.append(rs)

    def ost3(oh):
        for half in range(2):
            osb = ep.tile([128, 512], BF16, tag="osb3")
            nc.vector.tensor_mul(osb, o_ps3[:, bass.ts(half, 512)],
                                 rbc_sb[half])
            nc.sync.dma_start(
                out=out[oh * 128:(oh + 1) * 128,
                        qq3 * QB + half * 512: qq3 * QB + (half + 1) * 512],
                in_=osb,
            )
    ost3(0)
    pm(1)
    ost3(1)


_NC_CACHE = {}


def _build():
    if "nc" in _NC_CACHE:
        return _NC_CACHE["nc"]
    nc = bacc.Bacc("TRN2", target_bir_lowering=False, debug=False, num_devices=8)
    io = {
        "xb": nc.dram_tensor("xb", [C, N], BF16, kind="ExternalInput").ap(),
        "wq": nc.dram_tensor("wq", [C, DH], BF16, kind="ExternalInput").ap(),
        "wk": nc.dram_tensor("wk", [C, DH], BF16, kind="ExternalInput").ap(),
        "wv": nc.dram_tensor("wv", [C, DH], BF16, kind="ExternalInput").ap(),
        "wo": nc.dram_tensor("wo", [DH, C], BF16, kind="ExternalInput").ap(),
        "relw": nc.dram_tensor("relw", [DH, 127], BF16, kind="ExternalInput").ap(),
        "relh": nc.dram_tensor("relh", [DH, 127], BF16, kind="ExternalInput").ap(),
        "ih": nc.dram_tensor("ih", [64, N], BF16, kind="ExternalInput").ap(),
        "out": nc.dram_tensor("out", [C, N], BF16, kind="ExternalOutput").ap(),
    }
    with tile.TileContext(nc) as tc:
        _body(tc, io)
    nc.compile()
    _NC_CACHE["nc"] = nc
    return nc


_last_in_maps = None


def kernel(x, w_qkv, w_out, rel_height, rel_width):
    global _last_in_maps
    BF = ml_dtypes.bfloat16
    x = np.ascontiguousarray(x, np.float32)
    w_qkv = np.asarray(w_qkv, np.float32)
    w_out = np.asarray(w_out, np.float32)
    rel_height = np.asarray(rel_height, np.float32)
    rel_width = np.asarray(rel_width, np.float32)

    scale = np.float32(DH ** -0.5)
    ih_const = np.repeat(np.eye(64, dtype=np.float32), 64, axis=1).astype(BF)
    relw_t = np.ascontiguousarray(rel_width.T).astype(BF)
    relh_t = np.ascontiguousarray(rel_height.T).astype(BF)

    in_maps = []
    for g in range(8):
        b, i = divmod(g, HEADS)
        sl = slice(i * DH, (i + 1) * DH)
        in_maps.append({
            "xb": np.ascontiguousarray(x[b].reshape(C, N)).astype(BF),
            "wq": np.ascontiguousarray((w_qkv[i * DH:(i + 1) * DH] * scale).T).astype(BF),
            "wk": np.ascontiguousarray(w_qkv[C + i * DH: C + (i + 1) * DH].T).astype(BF),
            "wv": np.ascontiguousarray(w_qkv[2 * C + i * DH: 2 * C + (i + 1) * DH].T).astype(BF),
            "wo": np.ascontiguousarray(w_out[:, sl].T).astype(BF),
            "relw": relw_t,
            "relh": relh_t,
            "ih": ih_const,
        })

    _last_in_maps = in_maps
    nc = _build()
    res = run_bass_kernel_spmd(nc, in_maps, core_ids=list(range(8)))
    parts = [r["out"].astype(np.float32) for r in res.results]
    out = np.empty((B, C, N), np.float32)
    for b in range(B):
        out[b] = parts[4 * b] + parts[4 * b + 1] + parts[4 * b + 2] + parts[4 * b + 3]
        out[b] += x[b].reshape(C, N)
    return out.reshape(B, C, HH, WW)
